# revision 9
# baseline (speedup 1.0000x reference)
"""Trainium2 Bass kernel for nn_CgpHmmCell (HMM forward scan).

Reference computation (per batch row b):
    A  = softmax(transition_kernel, axis=-1)          # (5,5) row-stochastic
    Bm = softmax(emission_kernel, axis=-1)            # (5,4)
    E[b,t,s]   = sum_a inputs[b,t,a] * Bm[s,a]
    alpha[b,0] = [E[b,0,0], 0, 0, 0, 0]
    alpha[b,t] = E[b,t,:] * (alpha[b,t-1] @ A)
    output     = alpha  # (B, T, 5)

Key numerical fact exploited: every step multiplies alpha's L1 norm by at
most max_s E[b,t,s] <= max_a inputs[b,t,a] < 1 (A is row-stochastic, Bm rows
sum to 1).  alpha decays geometrically and underflows to exact fp32 zero
after ~130 steps for uniform[0,1) inputs.  The kernel computes a rigorous
per-batch upper bound on the live horizon T0 on the host (cheap numpy pass),
runs the serial scan only for t < T0, and zero-fills the rest of the output.

Sharding: data-parallel over batch, 8 NeuronCores x 256 rows each.

On-device layout (per core), with G=16 batch groups x bpg=16 rows:
    x_ga       [64=(a*G+g), bpg*T0]   input, free=(b',t)  (host pre-arranged)
    E_scan     [80=(g*5+s), bpg*T0]   emissions, free=(b',t)
    alpha_hist [80=(g*5+s), T0*bpg]   all scan states, free=(t,b')
    wb         [64, 80]  block-structured Bm: one matmul -> E for all groups
    wa         [80, 80]  block-diagonal A: one matmul advances all groups
Scan step t: matmul(psum = wa^T @ alpha[t-1]) ; alpha[t] = psum * E[:,:,t].
Output transposed back to (b,t,s) via PE transpose-mode matmuls.
"""

import numpy as np

import concourse.bacc as bacc
import concourse.bass as bass
import concourse.mybir as mybir
from concourse import tile
from concourse.bass_utils import run_bass_kernel_spmd

F32 = mybir.dt.float32

S = 5
AD = 4  # alphabet
N_CORES = 8


def _softmax(x, axis):
    x = x - x.max(axis=axis, keepdims=True)
    e = np.exp(x)
    return e / e.sum(axis=axis, keepdims=True)


def build_program(B_loc, T0, T_full, G, bpg, zero_fill=True, x_dma_chunks=4):
    """Build the per-core Bass program."""
    assert G * bpg == B_loc
    P5 = G * S       # scan partitions
    P4 = G * AD      # x partitions
    assert P5 <= 128 and P4 <= 128
    assert (T0 * bpg) % 128 == 0

    nc = bacc.Bacc("TRN2", target_bir_lowering=False)

    # consts packed into one tensor -> one DMA -> one wait semaphore.
    # layout along free dim: [wa (P5) | wb (P5) | smask (1) | ident (128)]
    CW_WA, CW_WB, CW_SM, CW_ID = 0, P5, 2 * P5, 2 * P5 + 1
    CW = 2 * P5 + 1 + 128
    consts = nc.dram_tensor("consts", [128, CW], F32, kind="ExternalInput")
    x = nc.dram_tensor("x", [P4, bpg * T0], F32, kind="ExternalInput")
    out = nc.dram_tensor("out", [B_loc, T_full, S], F32, kind="ExternalOutput")

    EMM_N = 512
    n_echunks = (bpg * T0 + EMM_N - 1) // EMM_N
    n_wchunks = (T0 * bpg) // 128
    ZCHUNK = 1920
    zlen_total = (T_full - T0) * S

    with tile.TileContext(nc) as tc:
        with (
            tc.tile_pool(name="const", bufs=1) as cpool,
            tc.tile_pool(name="xga", bufs=1) as xpool,
            tc.tile_pool(name="escan", bufs=1) as epool,
            tc.tile_pool(name="ahist", bufs=1) as apool,
            tc.tile_pool(name="ostage", bufs=1) as opool,
            tc.tile_pool(name="pe", bufs=2, space="PSUM") as pe_pool,
            tc.tile_pool(name="ps", bufs=4, space="PSUM") as ps_pool,
            tc.tile_pool(name="pt", bufs=2, space="PSUM") as pt_pool,
        ):
            ct = cpool.tile([128, CW], F32)
            nc.sync.dma_start(ct[:], consts[:])
            wa_t = ct[:P5, CW_WA:CW_WA + P5]
            wb_t = ct[:P4, CW_WB:CW_WB + P5]
            smask_t = ct[:P5, CW_SM:CW_SM + 1]
            id_t = ct[:P5, CW_ID:CW_ID + P5]

            zero_t = cpool.tile([128, ZCHUNK], F32)
            nc.gpsimd.memset(zero_t[:], 0.0)

            # ---- zero-fill the dead region t >= T0 ----
            if zero_fill:
                for b0 in range(0, B_loc, 128):
                    b1 = min(B_loc, b0 + 128)
                    done = 0
                    while done < zlen_total:
                        ln = min(ZCHUNK, zlen_total - done)
                        dst = out.ap()[b0:b1, :, :].rearrange(
                            "p t s -> p (t s)"
                        )[:, T0 * S + done: T0 * S + done + ln]
                        nc.sync.dma_start(dst, zero_t[: b1 - b0, :ln])
                        done += ln

            # ---- load x (host pre-arranged to [(a,g), (b', t)]) ----
            x_ga = xpool.tile([P4, bpg * T0], F32)
            # chunk along free so each E-matmul depends on one DMA
            cols = bpg * T0
            chunk = ((cols + x_dma_chunks - 1) // x_dma_chunks + EMM_N - 1) \
                // EMM_N * EMM_N
            for lo in range(0, cols, chunk):
                hi = min(cols, lo + chunk)
                nc.sync.dma_start(x_ga[:, lo:hi], x.ap()[:, lo:hi])

            # ---- E = Bm-matmul over all groups ----
            E_scan = epool.tile([P5, bpg * T0], F32)
            for c in range(n_echunks):
                lo = c * EMM_N
                hi = min(bpg * T0, lo + EMM_N)
                pe_t = pe_pool.tile([P5, EMM_N], F32)
                nc.tensor.matmul(pe_t[:, : hi - lo], wb_t, x_ga[:, lo:hi])
                nc.scalar.copy(E_scan[:, lo:hi], pe_t[:, : hi - lo])

            E3 = E_scan[:].rearrange("p (b t) -> p b t", b=bpg)

            # ---- scan ----
            alpha_hist = apool.tile([P5, T0 * bpg], F32)
            nc.vector.tensor_scalar(
                alpha_hist[:, 0:bpg],
                E3[:, :, 0],
                smask_t,
                None,
                mybir.AluOpType.mult,
            )
            for t in range(1, T0):
                ps_t = ps_pool.tile([P5, bpg], F32)
                nc.tensor.matmul(
                    ps_t[:], wa_t, alpha_hist[:, (t - 1) * bpg: t * bpg]
                )
                nc.vector.tensor_mul(
                    alpha_hist[:, t * bpg:(t + 1) * bpg], ps_t[:], E3[:, :, t]
                )

            # ---- transpose live region back and write out ----
            # alpha_hist free is (t, b'); 128 cols = tpc timesteps x bpg rows
            tpc = 128 // bpg
            ost = opool.tile([128, n_wchunks * P5], F32)
            for c in range(n_wchunks):
                pt_t = pt_pool.tile([128, P5], F32)
                nc.tensor.transpose(
                    pt_t[:], alpha_hist[:, c * 128:(c + 1) * 128], id_t
                )
                nc.scalar.copy(ost[:, c * P5:(c + 1) * P5], pt_t[:])
            # ost partition = (t', b'), free = (c, g, s)
            # DRAM addr: out[g*bpg + b', c*tpc + t', s]
            out_r = out.ap().rearrange(
                "(g b) (c t) s -> t g b c s", b=bpg, t=tpc
            )
            for tp in range(tpc):
                for g in range(G):
                    dst = out_r[tp][g][:, :n_wchunks, :]
                    src = ost[tp * bpg:(tp + 1) * bpg, :].rearrange(
                        "b (c g s) -> b c g s", c=n_wchunks, g=G
                    )[:, :, g, :]
                    nc.sync.dma_start(dst, src)

    nc.compile()
    return nc


def make_consts(G):
    P5, P4 = G * S, G * AD
    CW = 2 * P5 + 1 + 128
    return P5, P4, CW


def host_prep(inputs, tk, ek, G, bpg, T0):
    """Build constant tensor + per-core x in device layout."""
    P5, P4 = G * S, G * AD
    A = _softmax(np.asarray(tk, np.float32), -1)
    Bm = _softmax(np.asarray(ek, np.float32), -1)

    CW = 2 * P5 + 1 + 128
    consts = np.zeros((128, CW), dtype=np.float32)
    # wa[(g,k),(g,s)] = A[k,s] block diagonal
    for g in range(G):
        consts[g * S:(g + 1) * S, g * S:(g + 1) * S] = A
    # wb[(a*G+g),(g,s)] = Bm[s,a]
    for g in range(G):
        for a in range(AD):
            consts[a * G + g, P5 + g * S:P5 + (g + 1) * S] = Bm[:, a]
    # smask
    consts[0:P5:S, 2 * P5] = 1.0
    # identity (P5 x P5 is enough; pad region harmless)
    np.fill_diagonal(consts[:, 2 * P5 + 1: 2 * P5 + 1 + 128], 1.0)

    B = inputs.shape[0]
    B_loc = B // N_CORES
    xs = []
    for c in range(N_CORES):
        sl = inputs[c * B_loc:(c + 1) * B_loc, :T0, :]          # (B_loc,T0,4)
        # -> [(a, g), (b', t)]
        v = sl.reshape(G, bpg, T0, AD)
        v = v.transpose(3, 0, 1, 2).reshape(P4, bpg * T0)
        xs.append(np.ascontiguousarray(v, dtype=np.float32))
    return consts, xs


def _live_horizon(inputs):
    """Rigorous fp32 die-out bound: ||alpha_t||_1 <= prod_{tau<=t} max_a x[b,tau,a]."""
    m = inputs.max(axis=2)                      # (B, T)
    m = np.clip(m, 1e-30, None)
    lc = np.cumsum(np.log2(m, dtype=np.float32), axis=1)
    alive = (lc > -160.0).any(axis=0)           # (T,)
    idx = np.nonzero(~alive)[0]
    return int(idx[0]) if len(idx) else inputs.shape[1]


def kernel(inputs, transition_kernel, emission_kernel):
    inputs = np.ascontiguousarray(inputs, dtype=np.float32)
    B, T_full, _ = inputs.shape
    B_loc = B // N_CORES
    G, bpg = 16, 16
    assert G * bpg == B_loc

    T0 = _live_horizon(inputs) + 8
    T0 = int(min(T_full, ((T0 + 31) // 32) * 32))

    consts, xs = host_prep(inputs, transition_kernel, emission_kernel,
                           G, bpg, T0)
    nc = build_program(B_loc, T0, T_full, G, bpg)

    in_maps = [{"x": xs[c], "consts": consts} for c in range(N_CORES)]
    res = run_bass_kernel_spmd(nc, in_maps, list(range(N_CORES)))
    global LAST_RESULT
    LAST_RESULT = res
    outs = [res.results[c]["out"] for c in range(N_CORES)]
    return np.concatenate(outs, axis=0)


LAST_RESULT = None


# revision 14
# speedup vs baseline: 3.0777x; 3.0777x over previous
"""Trainium2 Bass kernel for nn_CgpHmmCell (HMM forward scan).

Reference computation (per batch row b):
    A  = softmax(transition_kernel, axis=-1)          # (5,5) row-stochastic
    Bm = softmax(emission_kernel, axis=-1)            # (5,4)
    E[b,t,s]   = sum_a inputs[b,t,a] * Bm[s,a]
    alpha[b,0] = [E[b,0,0], 0, 0, 0, 0]
    alpha[b,t] = E[b,t,:] * (alpha[b,t-1] @ A)
    output     = alpha  # (B, T, 5)

Numerical structure exploited: each step multiplies alpha's L1 norm by at
most max_s E[b,t,s] <= max_a inputs[b,t,a] < 1 (A row-stochastic, Bm rows
sum to 1), so alpha underflows to exact fp32 zero after ~130 steps for
uniform inputs.  The host computes a rigorous per-batch bound on the live
horizon T0 (cheap numpy pass); the device runs the scan only for t < T0.
The t >= T0 output region is exactly zero and is assembled on the host.

Sharding: data-parallel over batch, 8 NeuronCores x 256 rows each.

Device layout (per core), G=16 batch groups x bpg=16 rows:
    x_ga    [64=(a*G+g), T0*bpg]  input, free=(t,b')  (host pre-arranged)
    E_scan  [80=(g*5+s), T0*bpg]  emissions, free=(t,b')
    a_f32   [80=(g*5+s), (t_hi+1)*bpg]  fp32 alphas, t <= t_hi
    a_bf    [80=(g*5+s), T0*bpg]        bf16 alphas, t > t_hi
    wb      [64, 80]  block-structured Bm: one matmul -> E for all groups
    wa      [80, 80]  block-diagonal A:    one matmul advances all groups
Scan step t: matmul(psum = wa^T @ alpha[t-1]) ; alpha[t] = psum * E[:, t].

Precision: E and alpha0 are exact fp32.  For t <= t_hi (large alphas) the
scan matmul uses a Dekker-style compensated bf16 split (wa = wa_hi + wa_lo,
alpha = hi + lo; three bf16 matmuls accumulated in PSUM fp32) giving
~2^-17 per-step accuracy.  For t > t_hi alpha < ~1e-5, so plain bf16
matmuls and bf16 alpha storage contribute absmax-relative error < 1e-7.

Both alpha regions are DMA'd out raw (contiguous); the host transposes
into (b, t, s) and pastes into the zero-filled full output.
"""

import numpy as np
import ml_dtypes

import concourse.bacc as bacc
import concourse.bass as bass
import concourse.mybir as mybir
from concourse import tile
from concourse.bass_utils import run_bass_kernel_spmd

F32 = mybir.dt.float32
BF16 = mybir.dt.bfloat16

S = 5
AD = 4  # alphabet
N_CORES = 8
T_HI = 16  # steps using compensated hi/lo bf16 matmuls


def _softmax(x, axis):
    x = x - x.max(axis=axis, keepdims=True)
    e = np.exp(x)
    return e / e.sum(axis=axis, keepdims=True)


def build_program(B_loc, T0, G, bpg, t_hi=T_HI, x_dma_chunks=4):
    """Per-core Bass program. Device outputs raw alpha history, two dtypes."""
    assert G * bpg == B_loc
    P5 = G * S
    P4 = G * AD
    assert P5 <= 128 and P4 <= 128
    t_hi = min(t_hi, T0 - 1)

    nc = bacc.Bacc("TRN2", target_bir_lowering=False)

    # fp32 consts: [wb (P5 cols) | smask (1 col)]
    consts = nc.dram_tensor("consts", [128, P5 + 1], F32, kind="ExternalInput")
    # bf16 consts: [wa_hi (P5) | wa_lo (P5)]
    constb = nc.dram_tensor("constb", [128, 2 * P5], BF16, kind="ExternalInput")
    x = nc.dram_tensor("x", [P4, T0 * bpg], F32, kind="ExternalInput")
    out_f = nc.dram_tensor("out_f", [P5, (t_hi + 1) * bpg], F32,
                           kind="ExternalOutput")
    out_b = nc.dram_tensor("out_b", [P5, (T0 - t_hi - 1) * bpg], BF16,
                           kind="ExternalOutput")

    EMM_N = 512
    n_echunks = (T0 * bpg + EMM_N - 1) // EMM_N

    with tile.TileContext(nc) as tc:
        with (
            tc.tile_pool(name="const", bufs=1) as cpool,
            tc.tile_pool(name="xga", bufs=1) as xpool,
            tc.tile_pool(name="escan", bufs=1) as epool,
            tc.tile_pool(name="ahist", bufs=1) as apool,
            tc.tile_pool(name="hilo", bufs=4) as hpool,
            tc.tile_pool(name="pe", bufs=2, space="PSUM") as pe_pool,
            tc.tile_pool(name="ps", bufs=4, space="PSUM") as ps_pool,
        ):
            ct = cpool.tile([128, P5 + 1], F32)
            nc.sync.dma_start(ct[:], consts[:])
            wb_t = ct[:P4, 0:P5]
            smask_t = ct[:P5, P5:P5 + 1]
            cb = cpool.tile([128, 2 * P5], BF16)
            nc.sync.dma_start(cb[:], constb[:])
            wa_hi = cb[:P5, 0:P5]
            wa_lo = cb[:P5, P5:2 * P5]

            # ---- load x (host pre-arranged to [(a,g), (t, b')]) ----
            x_ga = xpool.tile([P4, T0 * bpg], F32)
            cols = T0 * bpg
            chunk = ((cols + x_dma_chunks - 1) // x_dma_chunks + EMM_N - 1) \
                // EMM_N * EMM_N
            for lo in range(0, cols, chunk):
                hi = min(cols, lo + chunk)
                nc.sync.dma_start(x_ga[:, lo:hi], x.ap()[:, lo:hi])

            # ---- E = Bm-matmul over all groups (fp32, exact) ----
            E_scan = epool.tile([P5, T0 * bpg], F32)
            for c in range(n_echunks):
                lo = c * EMM_N
                hi = min(T0 * bpg, lo + EMM_N)
                pe_t = pe_pool.tile([P5, EMM_N], F32)
                nc.tensor.matmul(pe_t[:, : hi - lo], wb_t, x_ga[:, lo:hi])
                nc.scalar.copy(E_scan[:, lo:hi], pe_t[:, : hi - lo])

            def E_t(t):
                return E_scan[:, t * bpg:(t + 1) * bpg]

            # ---- scan ----
            a_f32 = apool.tile([P5, (t_hi + 1) * bpg], F32, tag="af")
            # slot k of a_bf holds alpha at t = t_hi + k (slot 0 = bf16 seed)
            a_bf = apool.tile([P5, (T0 - t_hi) * bpg], BF16, tag="ab")

            nc.vector.tensor_scalar(
                a_f32[:, 0:bpg], E_t(0), smask_t, None, mybir.AluOpType.mult,
            )

            for t in range(1, T0):
                ps_t = ps_pool.tile([P5, bpg], F32)
                if t <= t_hi:
                    prev = a_f32[:, (t - 1) * bpg: t * bpg]
                    hi_sl = hpool.tile([P5, bpg], BF16, tag="hi")
                    lo_sl = hpool.tile([P5, bpg], BF16, tag="lo")
                    # hi = bf16(alpha); lo = bf16(alpha - fp32(hi))
                    nc.vector.tensor_copy(hi_sl[:], prev)
                    nc.vector.tensor_sub(lo_sl[:], prev, hi_sl[:])
                    nc.tensor.matmul(ps_t[:], wa_hi, hi_sl[:],
                                     start=True, stop=False)
                    nc.tensor.matmul(ps_t[:], wa_hi, lo_sl[:],
                                     start=False, stop=False)
                    nc.tensor.matmul(ps_t[:], wa_lo, hi_sl[:],
                                     start=False, stop=True)
                    cur = a_f32[:, t * bpg:(t + 1) * bpg]
                    nc.vector.tensor_mul(cur, ps_t[:], E_t(t))
                    if t == t_hi:
                        # boundary: bf16 seed for the plain-bf16 phase
                        nc.scalar.copy(a_bf[:, 0:bpg], cur)
                else:
                    prev_b = a_bf[:, (t - t_hi - 1) * bpg:(t - t_hi) * bpg]
                    nc.tensor.matmul(ps_t[:], wa_hi, prev_b)
                    nc.vector.tensor_mul(
                        a_bf[:, (t - t_hi) * bpg:(t - t_hi + 1) * bpg],
                        ps_t[:], E_t(t),
                    )

            nc.sync.dma_start(out_f.ap()[:], a_f32[:])
            nc.sync.dma_start(out_b.ap()[:], a_bf[:, bpg:])

    nc.compile()
    return nc


def host_prep(inputs, tk, ek, G, bpg, T0):
    """Build constant tensors + per-core x in device layout."""
    P5, P4 = G * S, G * AD
    A = _softmax(np.asarray(tk, np.float32), -1)
    Bm = _softmax(np.asarray(ek, np.float32), -1)

    consts = np.zeros((128, P5 + 1), dtype=np.float32)
    for g in range(G):
        for a in range(AD):
            consts[a * G + g, g * S:(g + 1) * S] = Bm[:, a]
    consts[0:P5:S, P5] = 1.0  # smask: 1 at s==0 partitions

    wa = np.zeros((P5, P5), dtype=np.float32)
    for g in range(G):
        wa[g * S:(g + 1) * S, g * S:(g + 1) * S] = A
    wa_hi = wa.astype(ml_dtypes.bfloat16)
    wa_lo = (wa - wa_hi.astype(np.float32)).astype(ml_dtypes.bfloat16)
    constb = np.zeros((128, 2 * P5), dtype=ml_dtypes.bfloat16)
    constb[:P5, 0:P5] = wa_hi
    constb[:P5, P5:2 * P5] = wa_lo

    B = inputs.shape[0]
    B_loc = B // N_CORES
    xs = []
    for c in range(N_CORES):
        sl = inputs[c * B_loc:(c + 1) * B_loc, :T0, :]          # (B_loc,T0,4)
        v = sl.reshape(G, bpg, T0, AD)
        v = v.transpose(3, 0, 2, 1).reshape(P4, T0 * bpg)       # [(a,g),(t,b')]
        xs.append(np.ascontiguousarray(v, dtype=np.float32))
    return consts, constb, xs


def _live_horizon(inputs):
    """Rigorous fp32 die-out bound: ||alpha_t||_1 <= prod_{tau<=t} max_a x[b,tau,a]."""
    m = inputs.max(axis=2)
    m = np.clip(m, 1e-30, None)
    lc = np.cumsum(np.log2(m, dtype=np.float32), axis=1)
    alive = (lc > -160.0).any(axis=0)
    idx = np.nonzero(~alive)[0]
    return int(idx[0]) if len(idx) else inputs.shape[1]


def kernel(inputs, transition_kernel, emission_kernel):
    inputs = np.ascontiguousarray(inputs, dtype=np.float32)
    B, T_full, _ = inputs.shape
    B_loc = B // N_CORES
    G, bpg = 16, 16
    assert G * bpg == B_loc

    T0 = _live_horizon(inputs) + 8
    T0 = int(min(T_full, ((T0 + 31) // 32) * 32))
    t_hi = min(T_HI, T0 - 1)

    consts, constb, xs = host_prep(inputs, transition_kernel,
                                   emission_kernel, G, bpg, T0)
    nc = build_program(B_loc, T0, G, bpg, t_hi=t_hi)

    in_maps = [{"x": xs[c], "consts": consts, "constb": constb}
               for c in range(N_CORES)]
    res = run_bass_kernel_spmd(nc, in_maps, list(range(N_CORES)))
    global LAST_RESULT
    LAST_RESULT = res

    full = np.zeros((B, T_full, S), dtype=np.float32)
    for c in range(N_CORES):
        af = np.asarray(res.results[c]["out_f"])         # [P5,(t_hi+1)*bpg] f32
        ab = np.asarray(res.results[c]["out_b"]).astype(np.float32)
        ah = np.concatenate(
            [af.reshape(P5_global(G), t_hi + 1, bpg),
             ab.reshape(P5_global(G), T0 - t_hi - 1, bpg)], axis=1,
        )                                                # (P5, T0, b')
        v = ah.reshape(G, S, T0, bpg).transpose(0, 3, 2, 1)   # (g,b',t,s)
        full[c * B_loc:(c + 1) * B_loc, :T0, :] = v.reshape(B_loc, T0, S)
    return full


def P5_global(G):
    return G * S


LAST_RESULT = None


# revision 18
# speedup vs baseline: 6.7110x; 2.1805x over previous
"""Trainium2 Bass kernel for nn_CgpHmmCell (HMM forward scan).

Reference computation (per batch row b):
    A  = softmax(transition_kernel, axis=-1)          # (5,5) row-stochastic
    Bm = softmax(emission_kernel, axis=-1)            # (5,4)
    E[b,t,s]   = sum_a inputs[b,t,a] * Bm[s,a]
    alpha[b,0] = [E[b,0,0], 0, 0, 0, 0]
    alpha[b,t] = E[b,t,:] * (alpha[b,t-1] @ A)
    output     = alpha  # (B, T, 5)

Numerical structure exploited: each step multiplies alpha's L1 norm by at
most max_s E[b,t,s] <= max_a inputs[b,t,a] < 1 (A row-stochastic, Bm rows
sum to 1), so alpha underflows to exact fp32 zero after ~130 steps for
uniform inputs.  The host computes a rigorous per-batch bound on the live
horizon T0 (cheap numpy pass); the device runs the scan only for t < T0.
The t >= T0 output region is exactly zero and is assembled on the host.

Sharding: data-parallel over batch, 8 NeuronCores x 256 rows each.

Device layout (per core), G=4 batch groups x bpg=64 rows (K=G*5=20 keeps
every matmul inside one 32-partition PE row-group -> exactly one
LDWEIGHTS+MATMUL pair per scan step):
    x_ga    [16=(a*G+g), T0*bpg]  input fp32 head / bf16 tail (host-split)
    E_scan  [20=(g*5+s), T0*bpg]  emissions, free=(t,b')
    a_f32   [20, (t_hi+1)*bpg]    fp32 alphas, t <= t_hi
    a_bf    [20, (T0-t_hi)*bpg]   bf16 alphas, t > t_hi
    wb      [16, 20]  block-structured Bm: one matmul -> E for all groups
    wa      [20, 20]  block-diagonal A:    one matmul advances all groups
Scan step t: matmul(psum = wa^T @ alpha[t-1]) ; alpha[t] = psum * E[:, t].

Precision: E and alpha0 are exact fp32 for t <= t_e (chunk-aligned past
t_hi); beyond that E uses plain bf16 matmuls (alpha there is < 1e-5, so
absolute error is ~1e-8 of the output scale).  For t <= t_hi the scan
matmul uses a Dekker-style compensated bf16 split (wa = wa_hi + wa_lo,
alpha = hi + lo; three bf16 matmuls accumulated in PSUM fp32) giving
~2^-17 per-step accuracy.  For t > t_hi plain bf16 matmuls + bf16 alpha
storage contribute absmax-relative error < 1e-7.

Both alpha regions are DMA'd out raw (contiguous); the host transposes
into (b, t, s) and pastes into the zero-filled full output.
"""

import numpy as np
import ml_dtypes

import concourse.bacc as bacc
import concourse.bass as bass
import concourse.mybir as mybir
from concourse import tile
from concourse.bass_utils import run_bass_kernel_spmd

F32 = mybir.dt.float32
BF16 = mybir.dt.bfloat16

S = 5
AD = 4  # alphabet
N_CORES = 8
T_HI = 16   # steps using compensated hi/lo bf16 scan matmuls
EMM_N = 512  # free elems per E matmul chunk


def _softmax(x, axis):
    x = x - x.max(axis=axis, keepdims=True)
    e = np.exp(x)
    return e / e.sum(axis=axis, keepdims=True)


def _t_split(T0, bpg, t_hi):
    """E/x fp32-vs-bf16 boundary, aligned to EMM chunks: first n_f chunks
    (covering at least t_hi+1 steps) stay fp32."""
    tpc = max(1, EMM_N // bpg)               # timesteps per E chunk
    n_chunks = (T0 + tpc - 1) // tpc
    n_f = min(n_chunks, (t_hi + 1 + tpc - 1) // tpc + 1)
    t_e = min(T0, n_f * tpc)                 # steps with fp32 E
    return tpc, n_chunks, n_f, t_e


def build_program(B_loc, T0, G, bpg, t_hi=T_HI):
    """Per-core Bass program. Device outputs raw alpha history, two dtypes."""
    assert G * bpg == B_loc
    P5 = G * S
    P4 = G * AD
    assert P5 <= 32 and P4 <= 32, "keep K inside one PE row-group"
    assert EMM_N % bpg == 0
    t_hi = min(t_hi, T0 - 1)
    tpc, n_chunks, n_f, t_e = _t_split(T0, bpg, t_hi)

    nc = bacc.Bacc("TRN2", target_bir_lowering=False)

    # fp32 consts: [wb (P5 cols) | smask (1 col)]
    consts = nc.dram_tensor("consts", [P5, P5 + 1], F32, kind="ExternalInput")
    # bf16 consts: [wa_hi (P5) | wa_lo (P5) | wb_bf (P5)]
    constb = nc.dram_tensor("constb", [P5, 3 * P5], BF16, kind="ExternalInput")
    xf = nc.dram_tensor("xf", [P4, t_e * bpg], F32, kind="ExternalInput")
    if T0 > t_e:
        xb = nc.dram_tensor("xb", [P4, (T0 - t_e) * bpg], BF16,
                            kind="ExternalInput")
    out_f = nc.dram_tensor("out_f", [P5, (t_hi + 1) * bpg], F32,
                           kind="ExternalOutput")
    out_b = nc.dram_tensor("out_b", [P5, (T0 - t_hi - 1) * bpg], BF16,
                           kind="ExternalOutput")

    with tile.TileContext(nc) as tc:
        with (
            tc.tile_pool(name="const", bufs=1) as cpool,
            tc.tile_pool(name="xga", bufs=1) as xpool,
            tc.tile_pool(name="escan", bufs=1) as epool,
            tc.tile_pool(name="ahist", bufs=1) as apool,
            tc.tile_pool(name="hilo", bufs=4) as hpool,
            tc.tile_pool(name="pe", bufs=2, space="PSUM") as pe_pool,
            tc.tile_pool(name="ps", bufs=4, space="PSUM") as ps_pool,
        ):
            ct = cpool.tile([P5, P5 + 1], F32)
            nc.sync.dma_start(ct[:], consts[:])
            wb_t = ct[:P4, 0:P5]
            smask_t = ct[:P5, P5:P5 + 1]
            cb = cpool.tile([P5, 3 * P5], BF16)
            nc.sync.dma_start(cb[:], constb[:])
            wa_hi = cb[:P5, 0:P5]
            wa_lo = cb[:P5, P5:2 * P5]
            wb_bf = cb[:P4, 2 * P5:3 * P5]

            # ---- load x (host pre-arranged to [(a,g), (t, b')]) ----
            x_f = xpool.tile([P4, t_e * bpg], F32, tag="xf")
            for lo in range(0, t_e * bpg, 4 * EMM_N):
                hi = min(t_e * bpg, lo + 4 * EMM_N)
                nc.sync.dma_start(x_f[:, lo:hi], xf.ap()[:, lo:hi])
            if T0 > t_e:
                x_b = xpool.tile([P4, (T0 - t_e) * bpg], BF16, tag="xb")
                nb = (T0 - t_e) * bpg
                for lo in range(0, nb, 8 * EMM_N):
                    hi = min(nb, lo + 8 * EMM_N)
                    nc.sync.dma_start(x_b[:, lo:hi], xb.ap()[:, lo:hi])

            # ---- E = Bm-matmul over all groups ----
            E_scan = epool.tile([P5, T0 * bpg], F32)
            for c in range(n_chunks):
                lo = c * EMM_N
                hi = min(T0 * bpg, lo + EMM_N)
                pe_t = pe_pool.tile([P5, EMM_N], F32)
                if c < n_f:
                    nc.tensor.matmul(pe_t[:, :hi - lo], wb_t, x_f[:, lo:hi])
                else:
                    nc.tensor.matmul(pe_t[:, :hi - lo], wb_bf,
                                     x_b[:, lo - t_e * bpg:hi - t_e * bpg])
                nc.scalar.copy(E_scan[:, lo:hi], pe_t[:, :hi - lo])

            def E_t(t):
                return E_scan[:, t * bpg:(t + 1) * bpg]

            # ---- scan ----
            a_f32 = apool.tile([P5, (t_hi + 1) * bpg], F32, tag="af")
            # slot k of a_bf holds alpha at t = t_hi + k (slot 0 = seed)
            a_bf = apool.tile([P5, (T0 - t_hi) * bpg], BF16, tag="ab")

            nc.vector.tensor_scalar(
                a_f32[:, 0:bpg], E_t(0), smask_t, None, mybir.AluOpType.mult,
            )

            for t in range(1, T0):
                ps_t = ps_pool.tile([P5, bpg], F32)
                if t <= t_hi:
                    prev = a_f32[:, (t - 1) * bpg: t * bpg]
                    hi_sl = hpool.tile([P5, bpg], BF16, tag="hi")
                    lo_sl = hpool.tile([P5, bpg], BF16, tag="lo")
                    # hi = bf16(alpha); lo = bf16(alpha - fp32(hi))
                    nc.vector.tensor_copy(hi_sl[:], prev)
                    nc.vector.tensor_sub(lo_sl[:], prev, hi_sl[:])
                    nc.tensor.matmul(ps_t[:], wa_hi, hi_sl[:],
                                     start=True, stop=False)
                    nc.tensor.matmul(ps_t[:], wa_hi, lo_sl[:],
                                     start=False, stop=False)
                    nc.tensor.matmul(ps_t[:], wa_lo, hi_sl[:],
                                     start=False, stop=True)
                    cur = a_f32[:, t * bpg:(t + 1) * bpg]
                    nc.vector.tensor_mul(cur, ps_t[:], E_t(t))
                    if t == t_hi:
                        nc.scalar.copy(a_bf[:, 0:bpg], cur)
                else:
                    prev_b = a_bf[:, (t - t_hi - 1) * bpg:(t - t_hi) * bpg]
                    nc.tensor.matmul(ps_t[:], wa_hi, prev_b)
                    nc.vector.tensor_mul(
                        a_bf[:, (t - t_hi) * bpg:(t - t_hi + 1) * bpg],
                        ps_t[:], E_t(t),
                    )

            nc.sync.dma_start(out_f.ap()[:], a_f32[:])
            nc.sync.dma_start(out_b.ap()[:], a_bf[:, bpg:])

    nc.compile()
    return nc


def host_prep(inputs, tk, ek, G, bpg, T0, t_hi):
    """Build constant tensors + per-core x in device layout."""
    P5, P4 = G * S, G * AD
    A = _softmax(np.asarray(tk, np.float32), -1)
    Bm = _softmax(np.asarray(ek, np.float32), -1)
    _, _, _, t_e = _t_split(T0, bpg, t_hi)

    consts = np.zeros((P5, P5 + 1), dtype=np.float32)
    wb = np.zeros((P4, P5), dtype=np.float32)
    for g in range(G):
        for a in range(AD):
            wb[a * G + g, g * S:(g + 1) * S] = Bm[:, a]
    consts[:P4, :P5] = wb
    consts[0:P5:S, P5] = 1.0  # smask: 1 at s==0 partitions

    wa = np.zeros((P5, P5), dtype=np.float32)
    for g in range(G):
        wa[g * S:(g + 1) * S, g * S:(g + 1) * S] = A
    wa_hi = wa.astype(ml_dtypes.bfloat16)
    wa_lo = (wa - wa_hi.astype(np.float32)).astype(ml_dtypes.bfloat16)
    constb = np.zeros((P5, 3 * P5), dtype=ml_dtypes.bfloat16)
    constb[:P5, 0:P5] = wa_hi
    constb[:P5, P5:2 * P5] = wa_lo
    constb[:P4, 2 * P5:3 * P5] = wb.astype(ml_dtypes.bfloat16)

    B = inputs.shape[0]
    B_loc = B // N_CORES
    xfs, xbs = [], []
    for c in range(N_CORES):
        sl = inputs[c * B_loc:(c + 1) * B_loc, :T0, :]          # (B_loc,T0,4)
        v = sl.reshape(G, bpg, T0, AD)
        v = v.transpose(3, 0, 2, 1).reshape(P4, T0 * bpg)       # [(a,g),(t,b')]
        xfs.append(np.ascontiguousarray(v[:, :t_e * bpg], dtype=np.float32))
        xbs.append(np.ascontiguousarray(
            v[:, t_e * bpg:]).astype(ml_dtypes.bfloat16))
    return consts, constb, xfs, xbs, t_e


def _live_horizon(inputs, Bm):
    """Rigorous fp32 die-out bound.

    A is row-stochastic so ||alpha @ A||_1 = ||alpha||_1, and
    ||alpha_t||_1 <= max_s E[b,t,s] * ||alpha_{t-1}||_1.  Once the log2 of
    the running product drops below -160 for every batch row, alpha is far
    below the smallest fp32 denormal and the reference output is exactly 0.
    Evaluated in growing prefixes so the host never touches most of T.
    """
    B, T, _ = inputs.shape
    hi = 512
    while True:
        hi = min(hi, T)
        e = np.einsum("bta,sa->bts", inputs[:, :hi, :], Bm,
                      dtype=np.float32)
        m = np.clip(e.max(axis=2), 1e-30, None)
        lc = np.cumsum(np.log2(m, dtype=np.float32), axis=1)
        alive = (lc > -160.0).any(axis=0)
        dead = np.nonzero(~alive)[0]
        if len(dead):
            return int(dead[0])
        if hi == T:
            return T
        hi *= 2


def kernel(inputs, transition_kernel, emission_kernel):
    inputs = np.ascontiguousarray(inputs, dtype=np.float32)
    B, T_full, _ = inputs.shape
    B_loc = B // N_CORES
    G, bpg = 4, 64
    assert G * bpg == B_loc
    P5 = G * S

    Bm = _softmax(np.asarray(emission_kernel, np.float32), -1)
    T0 = _live_horizon(inputs, Bm) + 8
    T0 = int(min(T_full, ((T0 + 31) // 32) * 32))
    t_hi = min(T_HI, T0 - 1)

    consts, constb, xfs, xbs, t_e = host_prep(
        inputs, transition_kernel, emission_kernel, G, bpg, T0, t_hi)
    nc = build_program(B_loc, T0, G, bpg, t_hi=t_hi)

    in_maps = []
    for c in range(N_CORES):
        m = {"xf": xfs[c], "consts": consts, "constb": constb}
        if T0 > t_e:
            m["xb"] = xbs[c]
        in_maps.append(m)
    res = run_bass_kernel_spmd(nc, in_maps, list(range(N_CORES)))
    global LAST_RESULT
    LAST_RESULT = res

    full = np.zeros((B, T_full, S), dtype=np.float32)
    for c in range(N_CORES):
        af = np.asarray(res.results[c]["out_f"])          # [P5,(t_hi+1)*bpg]
        ab = np.asarray(res.results[c]["out_b"]).astype(np.float32)
        ah = np.concatenate(
            [af.reshape(P5, t_hi + 1, bpg),
             ab.reshape(P5, T0 - t_hi - 1, bpg)], axis=1,
        )                                                 # (P5, T0, b')
        v = ah.reshape(G, S, T0, bpg).transpose(0, 3, 2, 1)
        full[c * B_loc:(c + 1) * B_loc, :T0, :] = v.reshape(B_loc, T0, S)
    return full


LAST_RESULT = None


# revision 22
# speedup vs baseline: 7.3238x; 1.0913x over previous
"""Trainium2 Bass kernel for nn_CgpHmmCell (HMM forward scan).

Reference computation (per batch row b):
    A  = softmax(transition_kernel, axis=-1)          # (5,5) row-stochastic
    Bm = softmax(emission_kernel, axis=-1)            # (5,4)
    E[b,t,s]   = sum_a inputs[b,t,a] * Bm[s,a]
    alpha[b,0] = [E[b,0,0], 0, 0, 0, 0]
    alpha[b,t] = E[b,t,:] * (alpha[b,t-1] @ A)
    output     = alpha  # (B, T, 5)

Numerical structure exploited: each step multiplies alpha's L1 norm by at
most max_s E[b,t,s] <= max_a inputs[b,t,a] < 1 (A row-stochastic, Bm rows
sum to 1), so alpha underflows to exact fp32 zero after ~130 steps for
uniform inputs.  The host computes a rigorous per-batch bound on the live
horizon T0 (cheap numpy pass); the device runs the scan only for t < T0.
The t >= T0 output region is exactly zero and is assembled on the host.

Sharding: data-parallel over batch, 8 NeuronCores x 256 rows each.

Device layout (per core), G=4 batch groups x bpg=64 rows (K=G*5=20 keeps
every matmul inside one 32-partition PE row-group -> exactly one
LDWEIGHTS+MATMUL pair per scan step):
    x_ga    [16=(a*G+g), T0*bpg]  input fp32 head / bf16 tail (host-split)
    E_scan  [20=(g*5+s), T0*bpg]  emissions, free=(t,b')
    a_f32   [20, (t_hi+1)*bpg]    fp32 alphas, t <= t_hi
    a_bf    [20, (T0-t_hi)*bpg]   bf16 alphas, t > t_hi
    wb      [16, 20]  block-structured Bm: one matmul -> E for all groups
    wa      [20, 20]  block-diagonal A:    one matmul advances all groups
Scan step t: matmul(psum = wa^T @ alpha[t-1]) ; alpha[t] = psum * E[:, t].

Precision: E and alpha0 are exact fp32 for t <= t_e (chunk-aligned past
t_hi); beyond that E uses plain bf16 matmuls (alpha there is < 1e-5, so
absolute error is ~1e-8 of the output scale).  For t <= t_hi the scan
matmul uses a Dekker-style compensated bf16 split (wa = wa_hi + wa_lo,
alpha = hi + lo; three bf16 matmuls accumulated in PSUM fp32) giving
~2^-17 per-step accuracy.  For t > t_hi plain bf16 matmuls + bf16 alpha
storage contribute absmax-relative error < 1e-7.

Both alpha regions are DMA'd out raw (contiguous); the host transposes
into (b, t, s) and pastes into the zero-filled full output.
"""

import numpy as np
import ml_dtypes

import concourse.bacc as bacc
import concourse.bass as bass
import concourse.mybir as mybir
from concourse import tile
from concourse.bass_utils import run_bass_kernel_spmd

F32 = mybir.dt.float32
BF16 = mybir.dt.bfloat16

S = 5
AD = 4  # alphabet
N_CORES = 8
T_HI = 16   # steps using compensated hi/lo bf16 scan matmuls
EMM_N = 512  # free elems per E matmul chunk


def _softmax(x, axis):
    x = x - x.max(axis=axis, keepdims=True)
    e = np.exp(x)
    return e / e.sum(axis=axis, keepdims=True)


def _t_split(T0, bpg, t_hi):
    """E/x fp32-vs-bf16 boundary, aligned to EMM chunks: first n_f chunks
    (covering at least t_hi+1 steps) stay fp32."""
    tpc = max(1, EMM_N // bpg)               # timesteps per E chunk
    n_chunks = (T0 + tpc - 1) // tpc
    n_f = min(n_chunks, (t_hi + 1 + tpc - 1) // tpc + 1)
    t_e = min(T0, n_f * tpc)                 # steps with fp32 E
    return tpc, n_chunks, n_f, t_e


def build_program(B_loc, T0, G, bpg, t_hi=T_HI):
    """Per-core Bass program. Device outputs raw alpha history, two dtypes."""
    assert G * bpg == B_loc
    P5 = G * S
    P4 = G * AD
    assert P5 <= 32 and P4 <= 32, "keep K inside one PE row-group"
    assert EMM_N % bpg == 0
    t_hi = min(t_hi, T0 - 1)
    tpc, n_chunks, n_f, t_e = _t_split(T0, bpg, t_hi)

    nc = bacc.Bacc("TRN2", target_bir_lowering=False)

    # fp32 consts: [wb (P5 cols) | smask (1 col)]
    consts = nc.dram_tensor("consts", [P5, P5 + 1], F32, kind="ExternalInput")
    # bf16 consts: [wa_hi (P5) | wa_lo (P5) | wb_bf (P5)]
    constb = nc.dram_tensor("constb", [P5, 3 * P5], BF16, kind="ExternalInput")
    xf = nc.dram_tensor("xf", [P4, t_e * bpg], F32, kind="ExternalInput")
    if T0 > t_e:
        xb = nc.dram_tensor("xb", [P4, (T0 - t_e) * bpg], BF16,
                            kind="ExternalInput")
    out_f = nc.dram_tensor("out_f", [P5, (t_hi + 1) * bpg], F32,
                           kind="ExternalOutput")
    out_b = nc.dram_tensor("out_b", [P5, (T0 - t_hi - 1) * bpg], BF16,
                           kind="ExternalOutput")

    with tile.TileContext(nc) as tc:
        with (
            tc.tile_pool(name="const", bufs=1) as cpool,
            tc.tile_pool(name="xga", bufs=1) as xpool,
            tc.tile_pool(name="escan", bufs=1) as epool,
            tc.tile_pool(name="ahist", bufs=1) as apool,
            tc.tile_pool(name="hilo", bufs=4) as hpool,
            tc.tile_pool(name="pe", bufs=2, space="PSUM") as pe_pool,
            tc.tile_pool(name="ps", bufs=4, space="PSUM") as ps_pool,
        ):
            ct = cpool.tile([P5, P5 + 1], F32)
            nc.sync.dma_start(ct[:], consts[:])
            wb_t = ct[:P4, 0:P5]
            smask_t = ct[:P5, P5:P5 + 1]
            cb = cpool.tile([P5, 3 * P5], BF16)
            nc.sync.dma_start(cb[:], constb[:])
            wa_hi = cb[:P5, 0:P5]
            wa_lo = cb[:P5, P5:2 * P5]
            wb_bf = cb[:P4, 2 * P5:3 * P5]

            # ---- load x (host pre-arranged to [(a,g), (t, b')]) ----
            x_f = xpool.tile([P4, t_e * bpg], F32, tag="xf")
            for lo in range(0, t_e * bpg, 4 * EMM_N):
                hi = min(t_e * bpg, lo + 4 * EMM_N)
                nc.sync.dma_start(x_f[:, lo:hi], xf.ap()[:, lo:hi])
            if T0 > t_e:
                x_b = xpool.tile([P4, (T0 - t_e) * bpg], BF16, tag="xb")
                nb = (T0 - t_e) * bpg
                for lo in range(0, nb, 8 * EMM_N):
                    hi = min(nb, lo + 8 * EMM_N)
                    nc.sync.dma_start(x_b[:, lo:hi], xb.ap()[:, lo:hi])

            # ---- E = Bm-matmul over all groups ----
            # first fp32 chunk is small so the scan can start ASAP
            E_scan = epool.tile([P5, T0 * bpg], F32)
            bounds = []
            lo = 0
            first = min(2 * bpg, t_e * bpg)
            if first:
                bounds.append((0, first, True))
                lo = first
            while lo < t_e * bpg:
                hi = min(t_e * bpg, lo + EMM_N)
                bounds.append((lo, hi, True))
                lo = hi
            while lo < T0 * bpg:
                hi = min(T0 * bpg, lo + EMM_N)
                bounds.append((lo, hi, False))
                lo = hi
            for lo, hi, is_f in bounds:
                pe_t = pe_pool.tile([P5, EMM_N], F32)
                if is_f:
                    nc.tensor.matmul(pe_t[:, :hi - lo], wb_t, x_f[:, lo:hi])
                else:
                    nc.tensor.matmul(pe_t[:, :hi - lo], wb_bf,
                                     x_b[:, lo - t_e * bpg:hi - t_e * bpg])
                nc.scalar.copy(E_scan[:, lo:hi], pe_t[:, :hi - lo])

            def E_t(t):
                return E_scan[:, t * bpg:(t + 1) * bpg]

            # ---- scan ----
            a_f32 = apool.tile([P5, (t_hi + 1) * bpg], F32, tag="af")
            # slot k of a_bf holds alpha at t = t_hi + k (slot 0 = seed)
            a_bf = apool.tile([P5, (T0 - t_hi) * bpg], BF16, tag="ab")

            nc.vector.tensor_scalar(
                a_f32[:, 0:bpg], E_t(0), smask_t, None, mybir.AluOpType.mult,
            )

            for t in range(1, T0):
                ps_t = ps_pool.tile([P5, bpg], F32)
                if t <= t_hi:
                    prev = a_f32[:, (t - 1) * bpg: t * bpg]
                    hi_sl = hpool.tile([P5, bpg], BF16, tag="hi")
                    lo_sl = hpool.tile([P5, bpg], BF16, tag="lo")
                    # hi = bf16(alpha); lo = bf16(alpha - fp32(hi))
                    nc.vector.tensor_copy(hi_sl[:], prev)
                    nc.vector.tensor_sub(lo_sl[:], prev, hi_sl[:])
                    nc.tensor.matmul(ps_t[:], wa_hi, hi_sl[:],
                                     start=True, stop=False)
                    nc.tensor.matmul(ps_t[:], wa_hi, lo_sl[:],
                                     start=False, stop=False)
                    nc.tensor.matmul(ps_t[:], wa_lo, hi_sl[:],
                                     start=False, stop=True)
                    cur = a_f32[:, t * bpg:(t + 1) * bpg]
                    nc.vector.tensor_mul(cur, ps_t[:], E_t(t))
                    if t == t_hi:
                        nc.scalar.copy(a_bf[:, 0:bpg], cur)
                else:
                    prev_b = a_bf[:, (t - t_hi - 1) * bpg:(t - t_hi) * bpg]
                    nc.tensor.matmul(ps_t[:], wa_hi, prev_b)
                    nc.vector.tensor_mul(
                        a_bf[:, (t - t_hi) * bpg:(t - t_hi + 1) * bpg],
                        ps_t[:], E_t(t),
                    )

            nc.sync.dma_start(out_f.ap()[:], a_f32[:])
            # stream the bf16 alpha history out in quarters so the DMA
            # overlaps the tail of the scan
            nb_out = (T0 - t_hi - 1) * bpg
            q = (nb_out // 4) // bpg * bpg
            lo = 0
            for piece in ([q, q, q] if q else []) + [nb_out - 3 * q]:
                if piece <= 0:
                    continue
                nc.sync.dma_start(out_b.ap()[:, lo:lo + piece],
                                  a_bf[:, bpg + lo:bpg + lo + piece])
                lo += piece

    nc.compile()
    return nc


def host_prep(inputs, tk, ek, G, bpg, T0, t_hi):
    """Build constant tensors + per-core x in device layout."""
    P5, P4 = G * S, G * AD
    A = _softmax(np.asarray(tk, np.float32), -1)
    Bm = _softmax(np.asarray(ek, np.float32), -1)
    _, _, _, t_e = _t_split(T0, bpg, t_hi)

    consts = np.zeros((P5, P5 + 1), dtype=np.float32)
    wb = np.zeros((P4, P5), dtype=np.float32)
    for g in range(G):
        for a in range(AD):
            wb[a * G + g, g * S:(g + 1) * S] = Bm[:, a]
    consts[:P4, :P5] = wb
    consts[0:P5:S, P5] = 1.0  # smask: 1 at s==0 partitions

    wa = np.zeros((P5, P5), dtype=np.float32)
    for g in range(G):
        wa[g * S:(g + 1) * S, g * S:(g + 1) * S] = A
    wa_hi = wa.astype(ml_dtypes.bfloat16)
    wa_lo = (wa - wa_hi.astype(np.float32)).astype(ml_dtypes.bfloat16)
    constb = np.zeros((P5, 3 * P5), dtype=ml_dtypes.bfloat16)
    constb[:P5, 0:P5] = wa_hi
    constb[:P5, P5:2 * P5] = wa_lo
    constb[:P4, 2 * P5:3 * P5] = wb.astype(ml_dtypes.bfloat16)

    B = inputs.shape[0]
    B_loc = B // N_CORES
    xfs, xbs = [], []
    for c in range(N_CORES):
        sl = inputs[c * B_loc:(c + 1) * B_loc, :T0, :]          # (B_loc,T0,4)
        v = sl.reshape(G, bpg, T0, AD)
        v = v.transpose(3, 0, 2, 1).reshape(P4, T0 * bpg)       # [(a,g),(t,b')]
        xfs.append(np.ascontiguousarray(v[:, :t_e * bpg], dtype=np.float32))
        xbs.append(np.ascontiguousarray(
            v[:, t_e * bpg:]).astype(ml_dtypes.bfloat16))
    return consts, constb, xfs, xbs, t_e


def _live_horizon(inputs, Bm):
    """Rigorous fp32 die-out bound.

    A is row-stochastic so ||alpha @ A||_1 = ||alpha||_1, and
    ||alpha_t||_1 <= max_s E[b,t,s] * ||alpha_{t-1}||_1.  Once the log2 of
    the running product drops below -160 for every batch row, alpha is far
    below the smallest fp32 denormal and the reference output is exactly 0.
    Evaluated in growing prefixes so the host never touches most of T.
    """
    B, T, _ = inputs.shape
    hi = 512
    while True:
        hi = min(hi, T)
        e = np.einsum("bta,sa->bts", inputs[:, :hi, :], Bm,
                      dtype=np.float32)
        m = np.clip(e.max(axis=2), 1e-30, None)
        lc = np.cumsum(np.log2(m, dtype=np.float32), axis=1)
        alive = (lc > -150.0).any(axis=0)
        dead = np.nonzero(~alive)[0]
        if len(dead):
            return int(dead[0])
        if hi == T:
            return T
        hi *= 2


def kernel(inputs, transition_kernel, emission_kernel):
    inputs = np.ascontiguousarray(inputs, dtype=np.float32)
    B, T_full, _ = inputs.shape
    B_loc = B // N_CORES
    G, bpg = 4, 64
    assert G * bpg == B_loc
    P5 = G * S

    Bm = _softmax(np.asarray(emission_kernel, np.float32), -1)
    T0 = _live_horizon(inputs, Bm) + 4
    T0 = int(min(T_full, ((T0 + 15) // 16) * 16))
    t_hi = min(T_HI, T0 - 1)

    consts, constb, xfs, xbs, t_e = host_prep(
        inputs, transition_kernel, emission_kernel, G, bpg, T0, t_hi)
    nc = build_program(B_loc, T0, G, bpg, t_hi=t_hi)

    in_maps = []
    for c in range(N_CORES):
        m = {"xf": xfs[c], "consts": consts, "constb": constb}
        if T0 > t_e:
            m["xb"] = xbs[c]
        in_maps.append(m)
    res = run_bass_kernel_spmd(nc, in_maps, list(range(N_CORES)))
    global LAST_RESULT
    LAST_RESULT = res

    full = np.zeros((B, T_full, S), dtype=np.float32)
    for c in range(N_CORES):
        af = np.asarray(res.results[c]["out_f"])          # [P5,(t_hi+1)*bpg]
        ab = np.asarray(res.results[c]["out_b"]).astype(np.float32)
        ah = np.concatenate(
            [af.reshape(P5, t_hi + 1, bpg),
             ab.reshape(P5, T0 - t_hi - 1, bpg)], axis=1,
        )                                                 # (P5, T0, b')
        v = ah.reshape(G, S, T0, bpg).transpose(0, 3, 2, 1)
        full[c * B_loc:(c + 1) * B_loc, :T0, :] = v.reshape(B_loc, T0, S)
    return full


LAST_RESULT = None
